# revision 52
# baseline (speedup 1.0000x reference)
"""Trainium2 Bass kernel for nn_NetV2 sparse CNN (submanifold sparse conv net).

Network: scatter 150 active pixels/image to 28x28 grid -> SubMConv3x3(1->32)+BN+ReLU
-> SubMConv3x3(32->64)+BN+ReLU -> SparseConv2x2s2(64->64)+BN+ReLU -> flatten(NCHW)
-> FC(12544->128)+ReLU -> FC(128->10) -> log_softmax.

Design notes:
  * The active-pixel pattern is identical for every image, so each sparse conv
    layer is a fixed gather+matmul structure shared batch-wide.
  * Batch is data-parallel across 8 cores (256 images/core) and lives on the
    matmul free (N) axis; channels/sites live on partitions.  BN folds into
    conv weights+bias.
  * Sites are matched into pairs; each pair owns one H2S block (site A on
    partitions 0-63, B on 64-127).  Pairs are chosen so that (a) cell-mates of
    a 2x2 pooling cell cohabit (conv3 handles the cell with ONE K=128 matmul)
    or (b) two single-cell sites cohabit (conv3 computes BOTH cells with one
    M=128 matmul), and so that the union of the pair's 3x3 neighbor sets fits
    in one 4-slot H1S block (conv2 handles the pair with ONE M=128 matmul).
  * H1S blocks hold up to 4 DISTINCT sites' conv1 outputs (32ch each); a
    site's conv2 operand contracts the whole block (K=128) with zero weight
    rows on slots it does not use.  Slots are shared between all consumers,
    which cuts conv1 matmuls and HBM traffic vs. per-consumer stacks.
  * conv1 emits each block with a single windowed matmul over a 32-aligned
    128-site window of the (row-major sorted) site vector X.
  * Every matmul is K=128 (fast-weight-load path) at tile_position (0,0) or
    (0,64); 2-bank PSUM supertiles hold 4 x 256-col groups and are evacuated
    by one fused relu+bias op, alternating scalar/vector engines.
  * FC2 keeps channels on partitions ([10,BC]) and does log-softmax with
    cross-partition sum via tiny ones-matmuls; output is [10,BC] (host
    transposes), so the final DMA is 10 fat descriptors instead of 256 thin.
  * PE warm-up matmuls + ACT-table preloads run during the input-DMA window.

All matmul operands are bf16 (fp32 PSUM accumulate); rel err ~2e-3 vs fp32 ref.
"""

import numpy as np
import ml_dtypes

B = 2048
S = 150          # active sites per image
H = W = 28
NCORES = 8
BC = B // NCORES  # batch per core = 256
EPS = 1e-5
BF = ml_dtypes.bfloat16
NWARM = 26       # PE warm-up matmuls (cover the HAM clock ramp window)

_CACHE = {}


# ---------------------------------------------------------------- metadata ---

def _build_meta(yy, xx):
    """Site pairing + packing metadata from the shared active-pixel pattern."""
    order = np.argsort(yy.astype(np.int64) * W + xx)  # row-major spatial sort
    yy_s, xx_s = yy[order], xx[order]
    grid = -np.ones((H, W), np.int64)
    grid[yy_s, xx_s] = np.arange(S)

    # 3x3 pad-1 neighbor lists: per out site i, list of (k, j), k ascending
    nbrs = []
    for i in range(S):
        y, x = int(yy_s[i]), int(xx_s[i])
        lst = []
        for ky in range(3):
            for kx in range(3):
                iy, ix = y + ky - 1, x + kx - 1
                if 0 <= iy < H and 0 <= ix < W and grid[iy, ix] >= 0:
                    lst.append((ky * 3 + kx, int(grid[iy, ix])))
        nbrs.append(lst)

    def nbr4(i):
        return [j for _, j in nbrs[i][:4]]

    def contributors(j):
        return [j2 for _, j2 in nbrs[j]]

    # 2x2 stride-2 cells: cell -> list of (k3, j)
    cellmap = {}
    for j in range(S):
        y, x = int(yy_s[j]), int(xx_s[j])
        cellmap.setdefault((y // 2, x // 2), []).append(((y % 2) * 2 + (x % 2), j))
    cells_xy = sorted(cellmap)
    C2 = len(cells_xy)
    site_cell = {}
    site_k3 = {}
    for c, cxy in enumerate(cells_xy):
        for k3, j in cellmap[cxy]:
            site_cell[j] = c
            site_k3[j] = k3

    # ---- H2S pairing ------------------------------------------------------
    # cell-mates first (conv3: one K=128 matmul per cell), then a max matching
    # on single-cell sites that maximizes conv2 co-packability.
    pairs = []        # (siteA, siteB): A -> half 0, B -> half 1
    pair_cellmate = []
    thirds = []       # third sites of 3-site cells
    singles = []
    for cxy in cells_xy:
        lst = cellmap[cxy]
        if len(lst) >= 2:
            pairs.append((lst[0][1], lst[1][1]))
            pair_cellmate.append(True)
            if len(lst) == 3:
                thirds.append(lst[2][1])
        else:
            singles.append(lst[0][1])

    def window_ok(content):
        lo, hi = min(content), max(content)
        return (hi // 32 - lo // 32) * 32 + 32 <= 128

    def copackable(a, b):
        U = set(nbr4(a)) | set(nbr4(b))
        if len(U) > 4 or len(nbrs[a]) > 4 or len(nbrs[b]) > 4:
            return False
        content = set()
        for u in U:
            content.update(contributors(u))
        return window_ok(content)

    try:
        import networkx as nx
        G = nx.Graph()
        G.add_nodes_from(singles)
        for ii, a in enumerate(singles):
            for b in singles[ii + 1:]:
                if copackable(a, b):
                    # prefer overlapping neighborhoods: smaller unions pack
                    # fewer H1S slots -> fewer conv1 blocks
                    w = 8 - len(set(nbr4(a)) | set(nbr4(b)))
                    G.add_edge(a, b, weight=w)
        matching = nx.max_weight_matching(G, maxcardinality=True)
        single_pairs = sorted({(min(a, b), max(a, b)) for a, b in matching})
    except ImportError:
        single_pairs = []
        used = set()
        for a in singles:
            if a in used:
                continue
            for b in singles:
                if b > a and b not in used and copackable(a, b):
                    single_pairs.append((a, b))
                    used.update((a, b))
                    break
    matched = {s for ab in single_pairs for s in ab}
    leftover = sorted([s for s in singles if s not in matched] + thirds)
    # pair leftovers (incl. m3 third sites) spatially
    while leftover:
        a = leftover.pop(0)
        b = leftover.pop(0) if leftover else None
        assert b is not None, "odd site count"
        single_pairs.append((a, b))
    for a, b in single_pairs:
        pairs.append((a, b))
        pair_cellmate.append(False)
    NB2 = len(pairs)
    # renumber pairs spatially so conv2 consumes H1S blocks in production order
    porder = sorted(range(NB2), key=lambda t: min(pairs[t]))
    pairs = [pairs[t] for t in porder]
    pair_cellmate = [pair_cellmate[t] for t in porder]
    site_place = {}
    for t, (a, b) in enumerate(pairs):
        site_place[a] = (t, 0)
        site_place[b] = (t, 1)

    # ---- H1S packing with slot sharing ------------------------------------
    blocks = []   # dict(sites=set, slots={idx: site}, content=set)
    site_h1 = {}  # site -> (block, slot) for each block that holds it

    def contribs_of(U):
        c = set()
        for u in U:
            c.update(contributors(u))
        return c

    def place_sites(U):
        """Put conv1 outputs of sites U into one block (sharing existing)."""
        Uc = contribs_of(U)
        best = None
        for bi, blk in enumerate(blocks):
            new = U - blk["sites"]
            if len(blk["sites"]) + len(new) > 4:
                continue
            if not window_ok(blk["content"] | Uc):
                continue
            score = (len(new), len(blk["content"] | Uc))
            if best is None or score < best[0]:
                best = (score, bi)
        if best is None:
            bi = len(blocks)
            blocks.append(dict(sites=set(), slots={}, content=set()))
        else:
            bi = best[1]
        blk = blocks[bi]
        for j in sorted(U - blk["sites"]):
            s = next(s for s in range(4) if s not in blk["slots"])
            blk["slots"][s] = j
            blk["sites"].add(j)
        blk["content"] |= Uc
        return bi

    # pairs already in spatial order; pack their H1S sources in that order
    conv2_src = {}    # pair t -> ('co', block) | ('sep', blockA, blockB)
    for t in range(NB2):
        a, b = pairs[t]
        if copackable(a, b):
            bi = place_sites(set(nbr4(a)) | set(nbr4(b)))
            conv2_src[t] = ("co", bi)
        else:
            ba = place_sites(set(nbr4(a)))
            bb = place_sites(set(nbr4(b)))
            conv2_src[t] = ("sep", ba, bb)

    # extra (5th) neighbors: reuse any block holding the site, else place
    extra_src = {}    # site -> (block, slot)
    for i in range(S):
        if len(nbrs[i]) == 5:
            j5 = nbrs[i][4][1]
            bi = next((b for b, blk in enumerate(blocks) if j5 in blk["sites"]),
                      None)
            if bi is None:
                bi = place_sites({j5})
            slot = next(s for s, j in blocks[bi]["slots"].items() if j == j5)
            extra_src[i] = (bi, slot)

    NB1 = len(blocks)
    kwin = []
    for blk in blocks:
        lo, hi = min(blk["content"]), max(blk["content"])
        a = lo // 32
        K = (hi // 32 - a + 1) * 32
        assert K <= 128
        kwin.append((a, K))

    # renumber blocks K-ascending within each DMA chunk: the operator is
    # shipped as two rects (rows 0:64 for all, rows 64:128 for the K>64
    # suffix), trimming ~25% of t1p HBM bytes
    remap = []
    for st, en in _t1chunks(NB1):
        remap += sorted(range(st, en), key=lambda b: kwin[b][1])
    inv = {old: new for new, old in enumerate(remap)}
    blocks = [blocks[old] for old in remap]
    kwin = [kwin[old] for old in remap]
    for t in list(conv2_src):
        src = conv2_src[t]
        conv2_src[t] = ("co", inv[src[1]]) if src[0] == "co" else \
            ("sep", inv[src[1]], inv[src[2]])
    extra_src = {s: (inv[b], sl) for s, (b, sl) in extra_src.items()}
    slot_of = [{j: s for s, j in blk["slots"].items()} for blk in blocks]

    # per-site slot map within its conv2 source block
    def site_slots(i, bi):
        return {j: slot_of[bi][j] for j in nbr4(i)}

    # ---- conv3 cell layout -------------------------------------------------
    # mp groups: one single-single pair -> 2 cells, one M=128 matmul
    # sd cells: cellmate-pair cells + orphan singles -> M=64 matmuls, 2/group
    mp_groups = []    # (pair t, cellA, cellB)
    sd_cells = []     # (cell, mainsrc, thirdsrc|None)
    for t, (a, b) in enumerate(pairs):
        if pair_cellmate[t]:
            c = site_cell[a]
            lst = cellmap[cells_xy[c]]
            third = None
            if len(lst) == 3:
                j3 = lst[2][1]
                t3, h3 = site_place[j3]
                third = (t3, h3, site_k3[j3])
            sd_cells.append((c, ("pair", t), third))
        else:
            for hf, s in enumerate((a, b)):
                cs = site_cell[s]
                if len(cellmap[cells_xy[cs]]) == 1:
                    sd_cells.append((cs, ("solo", t, hf, site_k3[s]), None))
            ca, cb = site_cell[a], site_cell[b]
            if (len(cellmap[cells_xy[ca]]) == 1
                    and len(cellmap[cells_xy[cb]]) == 1):
                mp_groups.append((t, ca, cb))
                sd_cells = sd_cells[:-2]  # covered by the mp group
    # groups: pack sd cells in pairs; order everything by max h2s block read
    # (conv2 produces blocks in index order) with any partial group last
    groups = []       # (descriptor, max_block, ncells)
    for t, ca, cb in mp_groups:
        a, b = pairs[t]
        groups.append((("mp", t, site_k3[a], site_k3[b], ca, cb), t, 2))

    def sd_block(entry):
        _c, mainsrc, third = entry
        tm = mainsrc[1]
        return max(tm, third[0]) if third else tm

    sd_cells.sort(key=sd_block)
    for gi in range(0, len(sd_cells), 2):
        pack = sd_cells[gi:gi + 2]
        groups.append((("sd", pack), max(sd_block(p) for p in pack),
                       len(pack)))
    groups.sort(key=lambda g: (g[2] < 2, g[1]))  # full first, by block dep
    cellseq = []
    for desc, _mb, _n in groups:
        if desc[0] == "mp":
            cellseq += [desc[4], desc[5]]
        else:
            cellseq += [c for c, _, _ in desc[1]]
    groups = [desc for desc, _mb, _n in groups]
    assert len(cellseq) == C2
    NB3 = len(groups)

    return dict(order=order, nbrs=nbrs, C2=C2, pairs=pairs,
                site_place=site_place, conv2_src=conv2_src,
                extra_src=extra_src, blocks=blocks, slot_of=slot_of,
                nb1=NB1, nb2=NB2, nb3=NB3, kwin=kwin,
                mp_groups=mp_groups, sd_cells=sd_cells, groups=groups,
                cellseq=cellseq, cells_xy=cells_xy, site_k3=site_k3,
                site_cell=site_cell)


# ----------------------------------------------------------- device program --

def _legalize_single_wait(bir_bytes):
    """Split instructions with >1 sem-wait into EventSemaphore + instruction.

    The walrus build in this environment supports a single sync-wait slot per
    instruction; Tile emits fused multi-waits. Carry the extra waits on
    standalone EventSemaphore instructions on the same engine (same semantics:
    the engine blocks in order until each condition passes).
    """
    import json as _json
    bir = _json.loads(bir_bytes)
    ctr = 0
    for fn in bir.get("functions", []):
        for blk in fn.get("blocks", []):
            insts = blk.get("instructions")
            if not insts:
                continue
            out = []
            for inst in insts:
                si = inst.get("sync_info")
                waits = (si or {}).get("on_wait") or []
                if len(waits) > 1:
                    for wt in waits[:-1]:
                        ctr += 1
                        out.append({
                            "debug": inst.get("debug", 0),
                            "engine": inst["engine"],
                            "ins": [], "outs": [],
                            "name": f"xw{ctr}_{inst['name']}",
                            "opcode": "EventSemaphore",
                            "sync_info": {"on_update": [], "on_wait": [wt]},
                        })
                    si["on_wait"] = [waits[-1]]
                out.append(inst)
            blk["instructions"] = out
    return _json.dumps(bir).encode()


# t1 operator DMA chunk sizes (blocks); first chunk small so conv1 starts early
def _t1chunks(NB1):
    sizes = [12, 18]
    out = []
    st = 0
    for s in sizes:
        if st + s >= NB1:
            break
        out.append((st, st + s))
        st += s
    out.append((st, NB1))
    return out


def _t1rects(meta):
    """Per chunk: (st, en, hi) - hi = first block (K-sorted) needing rows 64+."""
    kwin, NB1 = meta["kwin"], meta["nb1"]
    rects = []
    for st, en in _t1chunks(NB1):
        hi = en
        for b in range(st, en):
            if kwin[b][1] > 64:
                hi = b
                break
        rects.append((st, en, hi))
    return rects


def _build_program(meta):
    import os
    import concourse.bass as bass
    import concourse.mybir as mybir
    import concourse.tile as tile
    STAGES = int(os.environ.get("KSTAGES", "9"))

    class _Bass(bass.Bass):
        def to_json_bytes(self):
            return _legalize_single_wait(super().to_json_bytes())

    dt = mybir.dt
    f32, bf16 = dt.float32, dt.bfloat16
    Relu = mybir.ActivationFunctionType.Relu
    Exp = mybir.ActivationFunctionType.Exp
    Ln = mybir.ActivationFunctionType.Ln
    add_op = mybir.AluOpType.add
    max_op = mybir.AluOpType.max

    nbrs = meta["nbrs"]
    pairs, conv2_src = meta["pairs"], meta["conv2_src"]
    extra_src, site_place = meta["extra_src"], meta["site_place"]
    kwin, groups = meta["kwin"], meta["groups"]
    NB1, NB2, NB3, C2 = meta["nb1"], meta["nb2"], meta["nb3"], meta["C2"]
    t1r = _t1rects(meta)
    NT1 = len(t1r)
    WCH = (NB2 + 2) // 3   # w2p DMA chunk (pairs)

    # small bf16 weights: w3pp | w3cm | w3sx | fc2w  (w2x rides in w2p0)
    n_extra = len(extra_src)
    XW = n_extra * 64
    O_w3pp = 0
    O_w3cm = O_w3pp + 16 * 128
    O_w3sx = O_w3cm + 16 * 64
    O_f2w = O_w3sx + 8 * 64
    WTOT = O_f2w + 10

    nc = _Bass()
    p_xc = nc.declare_dram_parameter("xc", [128, 5 * BC], bf16, isOutput=False)
    p_t1lo = [nc.declare_dram_parameter(f"t1lo{q}", [64, (en - st) * 128], bf16,
                                        isOutput=False)
              for q, (st, en, hi) in enumerate(t1r)]
    p_t1hi = [nc.declare_dram_parameter(f"t1hi{q}", [64, (en - hi) * 128], bf16,
                                        isOutput=False) if en > hi else None
              for q, (st, en, hi) in enumerate(t1r)]
    p_w2p = [nc.declare_dram_parameter(
        f"w2p{q}", [128, (min(NB2, (q + 1) * WCH) - q * WCH) * 128
                    + (XW if q == 0 else 0)], bf16,
        isOutput=False) for q in range(3)]
    p_wp = nc.declare_dram_parameter("wpack", [128, WTOT], bf16, isOutput=False)
    p_f1 = nc.declare_dram_parameter("fc1g", [128, NB3 * 128], bf16, isOutput=False)
    # cpack cols: 0=b1t 1=b2t 2=b3t 3=fc1bt 4=fc2b(rows 0-9)
    p_cp = nc.declare_dram_parameter("cpack", [128, 5], f32, isOutput=False)
    p_out = nc.declare_dram_parameter("out", [10, BC], f32, isOutput=True)

    with tile.TileContext(nc) as tc:
        with (
            tc.tile_pool(name="consts", bufs=1) as consts,
            tc.tile_pool(name="acts", bufs=1) as acts,
            tc.tile_pool(name="pp", bufs=3, space=bass.MemorySpace.PSUM) as pp,
            tc.tile_pool(name="pfc", bufs=1, space=bass.MemorySpace.PSUM) as pfc,
            tc.tile_pool(name="small", bufs=2) as small,
        ):
            # ---- PE warm-up + ACT table preload during the DMA window -----
            wsrc = consts.tile([128, 256], bf16)
            nc.vector.memset(wsrc, 0.001)
            onesb = consts.tile([128, 16], bf16)
            nc.vector.memset(onesb, 1.0)
            wps = pp.tile([128, 1024], f32, tag="ps")
            for _ in range(NWARM):
                nc.tensor.matmul(wps[:, 0:256], wsrc[:, 0:128], wsrc,
                                 start=True, stop=True)
            wact = small.tile([128, 1], f32, tag="wact")
            nc.scalar.activation(out=wact, in_=wsrc[:, 0:1], func=Relu)
            nc.scalar.activation(out=wact, in_=wact, func=Exp)
            nc.scalar.activation(out=wact, in_=wact, func=Ln)

            xc = consts.tile([128, 5 * BC], bf16)
            t1p = [consts.tile([128, (t1r[q][1] - t1r[q][0]) * 128], bf16,
                               tag=f"t1p{q}", name=f"t1p{q}") for q in range(NT1)]
            w2pt = [consts.tile([128, (min(NB2, (q + 1) * WCH) - q * WCH) * 128
                                 + (XW if q == 0 else 0)],
                                bf16, tag=f"w2p{q}", name=f"w2p{q}")
                    for q in range(3)]
            wpack = consts.tile([128, WTOT], bf16)
            cpack = consts.tile([128, 5], f32)
            fc1g = consts.tile([128, NB3 * 128], bf16)

            fc2w = wpack[:, O_f2w:O_f2w + 10]
            b1t = cpack[:, 0:1]
            b2t = cpack[:, 1:2]
            b3t = cpack[:, 2:3]
            fc1bt = cpack[:, 3:4]
            fc2b10 = cpack[0:10, 4:5]

            h1s = acts.tile([128, NB1 * BC], bf16)
            h2s = acts.tile([128, NB2 * BC], bf16)
            h3s = acts.tile([128, NB3 * BC], bf16)
            zt = acts.tile([128, BC], bf16)

            # xc + first t1p chunk lead; tiny cpack only gates the first evac
            nc.sync.dma_start(out=xc, in_=p_xc[:])
            for q, (st, en, hi) in enumerate(t1r):
                nc.sync.dma_start(out=t1p[q][0:64, :], in_=p_t1lo[q][:])
                if en > hi:
                    nc.sync.dma_start(out=t1p[q][64:128, (hi - st) * 128:],
                                      in_=p_t1hi[q][:])
                if q == 0:
                    nc.sync.dma_start(out=cpack, in_=p_cp[:])
            for q in range(3):
                nc.sync.dma_start(out=w2pt[q], in_=p_w2p[q][:])
            nc.sync.dma_start(out=wpack, in_=p_wp[:])
            nc.sync.dma_start(out=fc1g, in_=p_f1[:])

            def evac(idx, dst, src, bias):
                # dst = relu(src + bias); alternate engines to split the load
                if idx % 2 == 0:
                    nc.scalar.activation(out=dst, in_=src, func=Relu,
                                         bias=bias, scale=1.0)
                else:
                    nc.vector.tensor_scalar(out=dst, in0=src, scalar1=bias,
                                            scalar2=0.0, op0=add_op, op1=max_op)

            def t1ap(b, K):
                """lhsT AP for conv1 block b from the chunked operator tiles."""
                for q in range(NT1):
                    if t1r[q][0] <= b < t1r[q][1]:
                        off = (b - t1r[q][0]) * 128
                        return t1p[q][0:K, off:off + 128]
                raise AssertionError

            # ---- conv1: H1S = relu(T1S^T @ Xwin + b1), one MM per block ---
            # K trimmed to the block's site window; 4 blocks per psum supertile
            # Filler warm-up matmuls at t1p chunk boundaries keep the PE busy
            # through DMA hiccups so the HAM clock ramp is never reset.
            pfw = pfc.tile([128, 512], f32, tag="pu")
            ei = 0
            for tp in range((NB1 + 3) // 4):
                if 4 * tp in (12, 28):
                    for _ in range(3):
                        nc.tensor.matmul(pfw[:, 0:256], wsrc[:, 0:128], wsrc,
                                         start=True, stop=True)
                bs = [b for b in range(4 * tp, 4 * tp + 4) if b < NB1]
                ps = pp.tile([128, 1024], f32, tag="ps")
                for ci, b in enumerate(bs):
                    a, K = kwin[b]
                    nc.tensor.matmul(ps[:, ci * 256:ci * 256 + 256],
                                     t1ap(b, K),
                                     xc[0:K, a * BC:(a + 1) * BC],
                                     start=True, stop=True,
                                     tile_position=(0, 0))
                evac(ei, h1s[:, bs[0] * BC:(bs[0] + len(bs)) * BC],
                     ps[:, 0:256 * len(bs)], b1t)
                ei += 1

            # ---- conv2: one matmul per co-packed pair, else one per site --
            if STAGES < 2:
                nc.vector.memset(h2s, 0.0)
            if STAGES < 3:
                nc.vector.memset(h3s, 0.0)

            def w2ap(t, off, width):
                q, rel = t // WCH, t % WCH
                return w2pt[q][:, rel * 128 + off:rel * 128 + off + width]

            def conv2_block(ps, t, ci):
                src = conv2_src[t]
                cols = ps[:, ci * 256:ci * 256 + 256]
                if src[0] == "co":
                    nc.tensor.matmul(cols, w2ap(t, 0, 128),
                                     h1s[:, src[1] * BC:(src[1] + 1) * BC],
                                     start=True, stop=True, tile_position=(0, 0))
                    return
                for hf, (site, bi) in enumerate(zip(pairs[t], src[1:])):
                    has_extra = site in extra_src
                    nc.tensor.matmul(
                        ps[64 * hf:64 * hf + 64, ci * 256:ci * 256 + 256],
                        w2ap(t, 64 * hf, 64),
                        h1s[:, bi * BC:(bi + 1) * BC],
                        start=True, stop=not has_extra,
                        tile_position=(0, 64 * hf))
                    if has_extra:
                        xe = sorted(extra_src).index(site)
                        xoff = WCH * 128 + xe * 64
                        nc.tensor.matmul(
                            ps[64 * hf:64 * hf + 64, ci * 256:ci * 256 + 256],
                            w2pt[0][:, xoff:xoff + 64],
                            h1s[:, extra_src[site][0] * BC:
                                (extra_src[site][0] + 1) * BC],
                            start=False, stop=True,
                            tile_position=(0, 64 * hf))

            if STAGES >= 2:
                for tp in range((NB2 + 3) // 4):
                    ts = [t for t in range(4 * tp, 4 * tp + 4) if t < NB2]
                    ps = pp.tile([128, 1024], f32, tag="ps")
                    for ci, t in enumerate(ts):
                        conv2_block(ps, t, ci)
                    evac(ei, h2s[:, ts[0] * BC:(ts[0] + len(ts)) * BC],
                         ps[:, 0:256 * len(ts)], b2t)
                    ei += 1

            # ---- conv3: M=128 for single-single groups, K=128 for pairs ---
            def kcmb(ka, kb):
                return ka * 4 + kb

            def conv3_group(ps, g, ci):
                """Emit group g into psum col group ci; return valid rows."""
                kind = groups[g]
                if kind[0] == "mp":
                    _, t, ka, kb = kind[:4]
                    nc.tensor.matmul(
                        ps[:, ci * 256:ci * 256 + 256],
                        wpack[:, O_w3pp + kcmb(ka, kb) * 128:
                              O_w3pp + kcmb(ka, kb) * 128 + 128],
                        h2s[:, t * BC:(t + 1) * BC],
                        start=True, stop=True, tile_position=(0, 0))
                    return 128
                _, pack = kind
                for hc, (cell, mainsrc, third) in enumerate(pack):
                    rows = ps[64 * hc:64 * hc + 64, ci * 256:ci * 256 + 256]
                    if mainsrc[0] == "pair":
                        t = mainsrc[1]
                        a, b = pairs[t]
                        ka, kb = meta["site_k3"][a], meta["site_k3"][b]
                        wap = wpack[:, O_w3cm + kcmb(ka, kb) * 64:
                                    O_w3cm + kcmb(ka, kb) * 64 + 64]
                    else:
                        _, t, hf, k3 = mainsrc
                        wap = wpack[:, O_w3sx + (hf * 4 + k3) * 64:
                                    O_w3sx + (hf * 4 + k3) * 64 + 64]
                    nc.tensor.matmul(rows, wap, h2s[:, t * BC:(t + 1) * BC],
                                     start=True, stop=third is None,
                                     tile_position=(0, 64 * hc))
                    if third is not None:
                        t3, h3, k3 = third
                        nc.tensor.matmul(
                            rows,
                            wpack[:, O_w3sx + (h3 * 4 + k3) * 64:
                                  O_w3sx + (h3 * 4 + k3) * 64 + 64],
                            h2s[:, t3 * BC:(t3 + 1) * BC],
                            start=False, stop=True,
                            tile_position=(0, 64 * hc))
                return 64 * len(pack)

            if STAGES >= 3:
                for tp in range((NB3 + 3) // 4):
                    gs = [g for g in range(4 * tp, 4 * tp + 4) if g < NB3]
                    ps = pp.tile([128, 1024], f32, tag="ps")
                    rws = [conv3_group(ps, g, ci) for ci, g in enumerate(gs)]
                    nfull = sum(1 for r in rws if r == 128)
                    assert all(r == 128 for r in rws[:nfull])
                    if nfull:
                        evac(ei, h3s[:, gs[0] * BC:(gs[0] + nfull) * BC],
                             ps[:, 0:256 * nfull], b3t)
                        ei += 1
                    for ci in range(nfull, len(gs)):
                        evac(ei, h3s[:rws[ci], gs[ci] * BC:(gs[ci] + 1) * BC],
                             ps[:rws[ci], ci * 256:ci * 256 + 256],
                             cpack[0:rws[ci], 2:3])
                        ei += 1

            # ---- FC1 + FC2 + log_softmax, two batch halves ----------------
            # u = fc2w^T @ z -> [10, .]; out = (u+b) - ln(sum_p exp(u+b));
            # half 1's FC1 matmuls hide half 0's softmax chain latency.
            # (no max-subtraction: logits are O(3), exp is safe in f32)
            psz = pfc.tile([128, BC], f32, tag="pz")
            psu = pfc.tile([128, 512], f32, tag="pu")
            ub = small.tile([128, BC], f32, tag="ub")
            e = small.tile([128, BC], bf16, tag="e")
            lse = small.tile([128, BC], bf16, tag="lse")
            o = small.tile([128, BC], f32, tag="o")
            for t in range(NB3):
                kt = min(128, C2 * 64 - t * 128)
                nc.tensor.matmul(psz, fc1g[:kt, t * 128:(t + 1) * 128],
                                 h3s[:kt, t * BC:(t + 1) * BC],
                                 start=(t == 0), stop=(t == NB3 - 1))
            nc.vector.tensor_scalar(out=zt, in0=psz, scalar1=fc1bt,
                                    scalar2=0.0, op0=add_op, op1=max_op)
            nc.tensor.matmul(psu[0:10, 0:BC], fc2w, zt, start=True, stop=True)
            nc.vector.tensor_scalar_add(ub[0:10], psu[0:10, 0:BC], fc2b10)
            nc.scalar.activation(out=e[0:10], in_=psu[0:10, 0:BC], func=Exp,
                                 bias=fc2b10, scale=1.0)
            nc.tensor.matmul(psu[0:1, 256:256 + BC], onesb[0:10, 0:1], e[0:10],
                             start=True, stop=True)
            nc.scalar.activation(out=lse[0:1], in_=psu[0:1, 256:256 + BC],
                                 func=Ln)
            psb = pfc.tile([128, BC], f32, tag="pz")
            nc.tensor.matmul(psb[0:10, 0:BC], onesb[0:1, 0:10], lse[0:1],
                             start=True, stop=True)
            nc.vector.tensor_sub(o[0:10], ub[0:10], psb[0:10, 0:BC])
            nc.sync.dma_start(out=p_out[:], in_=o[0:10])

    return nc


# ------------------------------------------------------------------- kernel --

def _fold_bn(w, g, b, m, v):
    s = np.asarray(g, np.float64) / np.sqrt(np.asarray(v, np.float64) + EPS)
    return (np.asarray(w, np.float64) * s).astype(np.float32), \
        (np.asarray(b, np.float64) - np.asarray(m, np.float64) * s).astype(np.float32)


def _host_arrays(meta, w1, g1, b1, m1, v1, w2, g2, b2, m2, v2,
                 w3, g3, b3, m3, v3, fc1_w, fc1_b, fc2_w, fc2_b):
    nbrs = meta["nbrs"]
    pairs, conv2_src = meta["pairs"], meta["conv2_src"]
    extra_src, kwin = meta["extra_src"], meta["kwin"]
    blocks, slot_of = meta["blocks"], meta["slot_of"]
    NB1, NB2, NB3, C2 = meta["nb1"], meta["nb2"], meta["nb3"], meta["C2"]
    M1 = NB1 * 128

    def nbr4(i):
        return nbrs[i][:4]

    w1f, t1 = _fold_bn(w1, g1, b1, m1, v1)
    w2f, t2 = _fold_bn(w2, g2, b2, m2, v2)
    w3f, t3 = _fold_bn(w3, g3, b3, m3, v3)

    # base conv1 operator columns per site: Tcols[src j, site, ch]
    w1k = w1f.reshape(9, 32)
    Tcols = np.zeros((S, S, 32), np.float32)
    for i in range(S):
        for k, j in nbrs[i]:
            Tcols[j, i] += w1k[k]

    # windowed stacked conv1 operator: block b's rows = sites [32a, 32a+128)
    T1P = np.zeros((128, M1), np.float32)
    for b, blk in enumerate(blocks):
        a, _K = kwin[b]
        for s, j in blk["slots"].items():
            cols = slice(b * 128 + s * 32, b * 128 + (s + 1) * 32)
            src = Tcols[32 * a: min(S, 32 * a + 128), j, :]
            T1P[:src.shape[0], cols] = src

    # conv2 operators: one 128-col block per pair
    w2k = w2f.reshape(9, 32, 64)
    w2p = np.zeros((128, NB2 * 128), np.float32)
    for t in range(NB2):
        src = conv2_src[t]
        for hf, site in enumerate(pairs[t]):
            bi = src[1] if src[0] == "co" else src[1 + hf]
            for k, j in nbr4(site):
                s = slot_of[bi][j]
                w2p[32 * s:32 * s + 32,
                    t * 128 + 64 * hf:t * 128 + 64 * hf + 64] = w2k[k]
    w2x = np.zeros((128, max(1, len(extra_src)) * 64), np.float32)
    for xe, site in enumerate(sorted(extra_src)):
        k5 = nbrs[site][4][0]
        _, slot = extra_src[site]
        w2x[32 * slot:32 * slot + 32, xe * 64:(xe + 1) * 64] = w2k[k5]

    # conv3 weight tables (shared by k3-combo)
    w3k = w3f.reshape(4, 64, 64)
    w3pp = np.zeros((128, 16 * 128), np.float32)
    w3cm = np.zeros((128, 16 * 64), np.float32)
    w3sx = np.zeros((128, 8 * 64), np.float32)
    for ka in range(4):
        for kb in range(4):
            c = ka * 4 + kb
            w3pp[0:64, c * 128:c * 128 + 64] = w3k[ka]
            w3pp[64:128, c * 128 + 64:c * 128 + 128] = w3k[kb]
            w3cm[0:64, c * 64:(c + 1) * 64] = w3k[ka]
            w3cm[64:128, c * 64:(c + 1) * 64] += w3k[kb]
    for hf in range(2):
        for k in range(4):
            w3sx[64 * hf:64 * hf + 64,
                 (hf * 4 + k) * 64:(hf * 4 + k + 1) * 64] = w3k[k]

    # FC1 rows gathered at active cells, cellseq order, K-chunked
    fc1_w = np.asarray(fc1_w, np.float32)
    cells_xy = meta["cells_xy"]
    rows = np.zeros((NB3 * 128, 128), np.float32)
    for nn_, c in enumerate(meta["cellseq"]):
        cy, cx = cells_xy[c]
        rows[nn_ * 64:(nn_ + 1) * 64] = fc1_w[np.arange(64) * 196 + cy * 14 + cx]
    fc1g = np.ascontiguousarray(
        rows.reshape(NB3, 128, 128).transpose(1, 0, 2).reshape(128, NB3 * 128))

    wpack = np.concatenate([
        w3pp, w3cm, w3sx,
        np.asarray(fc2_w, np.float32)], axis=1)
    cpack = np.zeros((128, 5), np.float32)
    cpack[:, 0] = np.tile(t1, 4)
    cpack[:, 1] = np.tile(t2, 2)
    cpack[:, 2] = np.tile(t3, 2)
    cpack[:, 3] = np.asarray(fc1_b, np.float32)
    cpack[0:10, 4] = np.asarray(fc2_b, np.float32)
    arrs = {
        "wpack": wpack.astype(BF),
        "fc1g": fc1g.astype(BF),
        "cpack": cpack,
    }
    WCH = (NB2 + 2) // 3
    w2pb = w2p.astype(BF)
    for q in range(3):
        cols = w2pb[:, q * WCH * 128:min(NB2, (q + 1) * WCH) * 128]
        if q == 0:  # w2x (extra-neighbor operators) rides in chunk 0
            cols = np.concatenate([cols, w2x.astype(BF)], axis=1)
        arrs[f"w2p{q}"] = np.ascontiguousarray(cols)
    # chunked t1 operator, two rects per chunk (rows 0:64 / 64:128 suffix)
    T1Pb = T1P.astype(BF)
    for q, (st, en, hi) in enumerate(_t1rects(meta)):
        arrs[f"t1lo{q}"] = np.ascontiguousarray(T1Pb[0:64, st * 128:en * 128])
        if en > hi:
            arrs[f"t1hi{q}"] = np.ascontiguousarray(
                T1Pb[64:128, hi * 128:en * 128])
    return arrs


def kernel(features, indices, batch_size, w1, g1, b1, m1, v1,
           w2, g2, b2, m2, v2, w3, g3, b3, m3, v3,
           fc1_w, fc1_b, fc2_w, fc2_b, _trace=False):
    from concourse.bass_utils import run_bass_kernel_spmd

    features = np.asarray(features, np.float32)
    indices = np.asarray(indices, np.int32)
    assert int(batch_size) == B and features.shape[0] == B * S

    assert np.array_equal(indices[:, 0], np.repeat(np.arange(B, dtype=np.int32), S)), \
        "indices must be batch-major"
    assert np.array_equal(indices[:, 1:].reshape(B, S, 2),
                          np.broadcast_to(indices[:S, 1:], (B, S, 2))), \
        "active pattern must be identical across the batch"

    yy, xx = indices[:S, 1].copy(), indices[:S, 2].copy()
    key = (yy.tobytes(), xx.tobytes())
    if key not in _CACHE:
        meta = _build_meta(yy, xx)
        _CACHE[key] = (meta, _build_program(meta))
    meta, nc = _CACHE[key]

    common = _host_arrays(meta, w1, g1, b1, m1, v1, w2, g2, b2, m2, v2,
                          w3, g3, b3, m3, v3, fc1_w, fc1_b, fc2_w, fc2_b)

    # X replicated at five 32-site alignments: copy a = sites [32a, 32a+128)
    XT = features.reshape(B, S)[:, meta["order"]].T  # [S, B]
    Xpad = np.zeros((32 * 4 + 128, B), np.float32)
    Xpad[:S] = XT
    in_maps = []
    for c in range(NCORES):
        m = dict(common)
        xcs = np.zeros((128, 5 * BC), np.float32)
        for a in range(5):
            xcs[:, a * BC:(a + 1) * BC] = Xpad[32 * a:32 * a + 128,
                                               c * BC:(c + 1) * BC]
        m["xc"] = xcs.astype(BF)
        in_maps.append(m)

    res = run_bass_kernel_spmd(nc, in_maps, list(range(NCORES)), trace=_trace)
    global LAST_RESULT
    LAST_RESULT = res
    # device emits [10, BC] (channels on partitions); transpose per core
    out = np.concatenate([np.asarray(res.results[c]["out"]).T
                          for c in range(NCORES)], axis=0)
    return np.asarray(out, np.float32)


LAST_RESULT = None
